# revision 9
# baseline (speedup 1.0000x reference)
"""Bass/Tile kernel for a single attention head, data-parallel over B=8 on
8 TRN2 NeuronCores (one batch element per core, no collectives).

Per-core problem (S=2048, D=1024, H=128):
    q = Xq @ Wq + bq ; k = Xk @ Wk + bk ; v = Xv @ Wv + bv
    out = softmax(q k^T / sqrt(H)) v

v2 layout strategy (PE contracts over the partition dim):
  - X^T is built ON THE HOST (numpy transpose + bf16 cast + repack), so
    the PE spends zero cycles transposing inputs (v1 burned ~22us there).
    Inputs arrive pre-packed so every DMA has 8KB contiguous per-partition
    lines: q/v as [quarter, p, chunk, 512], k as [quarter, p, chunk, 512],
    weights as [p, chunk, 128] bf16.
  - q^T/k^T quarters [h, s] from stationary W d-chunks (bias fused into
    the DVE PSUM drain as a per-partition scalar).
  - scoresT [j, i] per j-tile (stationary k^T slice, moving q^T quarters);
    exp((1/sqrt(H)) x) is one ACT pass per (j-tile, i-half) PSUM->SBUF
    bf16.  The ACT engine is the serial bottleneck (~37us of exp) so the
    whole schedule is built to start it ASAP and keep it saturated: k
    streams in quarters right behind the first q half.
  - v is projected DIRECTLY to natural [s, h] (stationary X_v^T s-slice,
    moving Wv chunk), bias added by a 1-partition ones-row matmul into
    the same PSUM group; a ones column makes AV emit numerator + softmax
    row-sums in one accumulation.
  - AV runs in 4 j-phases chasing the exp stream, accumulating i-tiles in
    SBUF f32 (PSUM is only 8 banks), so only the last j-phase (~4us)
    trails the final exp.  Normalize = DVE reciprocal + ACT copy w/
    per-partition scale (ACT is idle after exp).
  - exp/v/acc tiles are split per j-quarter & i-half so the Tile
    whole-tile dependency tracking never over-serializes the pipeline.
"""

import sys

if "/opt/trn_rl_repo" not in sys.path:
    sys.path.insert(0, "/opt/trn_rl_repo")

import numpy as np

import concourse.bass as bass
import concourse.tile as tile
from concourse import bacc, mybir
from concourse.bass_utils import run_bass_kernel_spmd

P = 128          # partitions
S = 2048         # sequence length (per core)
D = 1024         # input dim
H = 128          # head dim (Dq = Dk)
ST = S // P      # 16 s-tiles
DC = D // P      # 8 d-chunks
NQ = 4           # s-quarters
QS = S // NQ     # 512
N_CORES = 8

F32 = mybir.dt.float32
BF16 = mybir.dt.bfloat16
AF = mybir.ActivationFunctionType

SOFTMAX_SCALE = 1.0 / float(np.sqrt(H))


def _build_kernel(tc, ins, out_ap):
    nc = tc.nc
    (qp, kp, vp, wq_ap, bq_ap, wk_ap, bk_ap, wv_ap, bv_ap) = ins

    with (
        tc.tile_pool(name="consts", bufs=1) as consts,
        tc.tile_pool(name="xq", bufs=4) as xqp,
        tc.tile_pool(name="xk", bufs=4) as xkp,
        tc.tile_pool(name="xv", bufs=4) as xvp,
        tc.tile_pool(name="proj", bufs=1) as projp,
        tc.tile_pool(name="expp", bufs=1) as expp,
        tc.tile_pool(name="vext", bufs=1) as vexp,
        tc.tile_pool(name="accp", bufs=1) as accp,
        tc.tile_pool(name="outp", bufs=4) as outp,
    ):
        # ---- tiny consts (no DMA) ----
        warm_a = consts.tile([P, P], BF16, tag="warm_a")
        nc.gpsimd.memset(warm_a, 0.5)
        ones_row = consts.tile([P, P], BF16, tag="ones_row")
        nc.gpsimd.memset(ones_row, 1.0)
        # preload the ACT exp table set during DMA dead time (~2.7us)
        dummy = consts.tile([P, 1], F32, tag="dummy")
        nc.gpsimd.memset(dummy, 0.0)
        exp_sink = consts.tile([P, 1], BF16, tag="exp_sink")
        nc.scalar.activation(exp_sink, dummy, AF.Exp, bias=0.0, scale=1.0)
        warm_sink = nc.dram_tensor("warm_sink", [P, P], F32)

        # ---- weights/biases + all input loads, in priority order ----
        # (dma_start issue order == sync-engine order == execution order)
        # Every staging tag has enough bufs that NO load DMA waits on a
        # compute-freed slot: all doorbells ring up front and the engines
        # stream the full 13.4MB at aggregate bandwidth in this priority
        # order (q half0 + k feed the exp stream; v is only needed last).
        wq = consts.tile([P, DC, H], BF16, tag="wq")
        nc.sync.dma_start(out=wq, in_=wq_ap)
        bq = consts.tile([P, 1], F32, tag="bq")
        nc.sync.dma_start(out=bq, in_=bq_ap)
        xq_tiles = []
        for nq in range(2):
            xt = xqp.tile([P, DC, QS], BF16, tag="xq", name=f"xq{nq}")
            nc.sync.dma_start(out=xt, in_=qp[nq])
            xq_tiles.append(xt)
        wk = consts.tile([P, DC, H], BF16, tag="wk")
        nc.sync.dma_start(out=wk, in_=wk_ap)
        bk = consts.tile([P, 1], F32, tag="bk")
        nc.sync.dma_start(out=bk, in_=bk_ap)
        xk_tiles = []
        for nq in range(NQ):
            xt = xkp.tile([P, DC, QS], BF16, tag="xk", name=f"xk{nq}")
            nc.sync.dma_start(out=xt, in_=kp[nq])
            xk_tiles.append(xt)
        for nq in range(2, NQ):
            xt = xqp.tile([P, DC, QS], BF16, tag="xq", name=f"xq{nq}")
            nc.sync.dma_start(out=xt, in_=qp[nq])
            xq_tiles.append(xt)
        wv = consts.tile([P, DC, H], BF16, tag="wv")
        nc.sync.dma_start(out=wv, in_=wv_ap)
        bvr = consts.tile([1, H], BF16, tag="bvr")
        nc.sync.dma_start(out=bvr, in_=bv_ap)
        xv_tiles = []
        for nq in range(NQ):
            xt = xvp.tile([P, DC, QS], BF16, tag="xv", name=f"xv{nq}")
            nc.sync.dma_start(out=xt, in_=vp[nq])
            xv_tiles.append(xt)

        # ---- persistent SBUF tiles ----
        qTq = [
            projp.tile([P, QS], BF16, tag=f"qT{i}", name=f"qT{i}")
            for i in range(NQ)
        ]
        kTq = [
            projp.tile([P, QS], BF16, tag=f"kT{i}", name=f"kT{i}")
            for i in range(NQ)
        ]
        # exp tiles split by (i-half, j-quarter) so AV j-phases only wait
        # on the exp they actually read
        ex = [
            [
                expp.tile([P, 4, 1024], BF16, tag=f"ex{h}{jq}", name=f"ex{h}{jq}")
                for jq in range(NQ)
            ]
            for h in range(2)
        ]
        # v natural [s,h]+ones column, split by j-quarter
        vx = [
            vexp.tile([P, 4, H + 1], BF16, tag=f"vx{jq}", name=f"vx{jq}")
            for jq in range(NQ)
        ]
        for jq in range(NQ):
            nc.gpsimd.memset(vx[jq][:, :, H : H + 1], 1.0)
        acc = [
            accp.tile([P, H + 1], F32, tag=f"acc{it}", name=f"acc{it}")
            for it in range(ST)
        ]
        # packed output halves: [p, it, h] with 4KB per-partition DMA
        # lines (a [128,128] f32 store has 512B lines and crawls)
        out_sb = [
            outp.tile([P, 8, H], F32, tag=f"osb{hf}", name=f"osb{hf}", bufs=1)
            for hf in range(2)
        ]

        with (
            tc.tile_pool(name="psS", bufs=2, space="PSUM") as psS,   # 2x2 banks
            tc.tile_pool(name="psV", bufs=2, space="PSUM") as psV,   # 2x1 banks
        ):
            with tc.tile_pool(name="psP", bufs=2, space="PSUM") as psP:  # 2x1
                # ---- PE warm-up: keep PE busy pre-data so the HAM clock
                # gate releases to 2.4GHz before the real stream ----
                ps_w = psP.tile([P, QS], F32, tag="pp", name="ps_w")
                for _ in range(64):
                    nc.tensor.matmul(
                        ps_w[:, 0:P], warm_a, warm_a, start=True, stop=True
                    )
                warm_sb = consts.tile([P, P], F32, tag="warm_sb")
                nc.vector.tensor_copy(warm_sb, ps_w[:, 0:P])
                nc.sync.dma_start(out=warm_sink[:, :], in_=warm_sb)

                def proj_quarter(xt, w, b, dst):
                    """dst[h, s_quarter] = (X^T chunk-contracted with W) + b."""
                    ps = psP.tile([P, QS], F32, tag="pp")
                    for dc in range(DC):
                        nc.tensor.matmul(
                            ps,
                            w[:, dc, :],
                            xt[:, dc, :],
                            start=(dc == 0),
                            stop=(dc == DC - 1),
                        )
                    nc.vector.tensor_scalar_add(dst, ps, b)

                def scores_exp(jt, hf):
                    """scoresT[j-tile jt, i-half hf] -> exp -> ex tile."""
                    kt_sl = kTq[jt // 4][:, (jt % 4) * P : (jt % 4 + 1) * P]
                    pss = psS.tile([P, 1024], F32, tag="ps")
                    for nb in range(2):
                        nc.tensor.matmul(
                            pss[:, nb * QS : (nb + 1) * QS],
                            kt_sl,
                            qTq[2 * hf + nb],
                            start=True,
                            stop=True,
                        )
                    nc.scalar.activation(
                        ex[hf][jt // 4][:, jt % 4, :],
                        pss,
                        AF.Exp,
                        bias=0.0,
                        scale=SOFTMAX_SCALE,
                    )

                # q half 0, then k quarters (scores i-half0 chase them)
                proj_quarter(xq_tiles[0], wq, bq, qTq[0])
                proj_quarter(xq_tiles[1], wq, bq, qTq[1])
                for kq in range(NQ):
                    proj_quarter(xk_tiles[kq], wk, bk, kTq[kq])
                    for jt in range(kq * 4, kq * 4 + 4):
                        scores_exp(jt, 0)
                proj_quarter(xq_tiles[2], wq, bq, qTq[2])
                proj_quarter(xq_tiles[3], wq, bq, qTq[3])
            # psP closed: its 2 banks are free (last reader is the q3
            # drain, NOT the exp tail) -> psB can take them safely.

            # scores i-half1 (ACT-paced via the psS rotation)
            for jt in range(ST):
                scores_exp(jt, 1)

            def vproj_quarter(jq):
                """v natural [s,h] for s-tiles 4jq..4jq+3, bias via a
                1-partition ones-row matmul into the same PSUM group."""
                xt = xv_tiles[jq]
                for st in range(4):
                    ps = psV.tile([P, H], F32, tag="pv")
                    for dc in range(DC):
                        nc.tensor.matmul(
                            ps,
                            xt[:, dc, st * P : (st + 1) * P],
                            wv[:, dc, :],
                            start=(dc == 0),
                            stop=False,
                        )
                    nc.tensor.matmul(
                        ps, ones_row[0:1, :], bvr[0:1, :], start=False, stop=True
                    )
                    nc.vector.tensor_copy(vx[jq][:, st, 0:H], ps)

            with tc.tile_pool(name="psB", bufs=2, space="PSUM") as psB:

                def av_phase(jp):
                    """AV partial products for j-tiles 4jp..4jp+3, all 16
                    i-tiles, accumulated into acc (SBUF f32).  Drains
                    alternate DVE/GpSimd so neither paces the PE."""
                    for it in range(ST):
                        po = psB.tile([P, H + 1], F32, tag="po")
                        for j in range(4):
                            jt = 4 * jp + j
                            nc.tensor.matmul(
                                po,
                                ex[it // 8][jp][:, j, (it % 8) * P : (it % 8 + 1) * P],
                                vx[jp][:, j, :],
                                start=(j == 0),
                                stop=(j == 3),
                            )
                        # GpSimd can't read PSUM, so all drains ride DVE
                        if jp == 0:
                            nc.vector.tensor_copy(acc[it], po)
                        else:
                            nc.vector.tensor_add(acc[it], acc[it], po)
                        if jp == 3:
                            rc = outp.tile([P, 1], F32, tag="rc")
                            nc.vector.reciprocal(rc, acc[it][:, H : H + 1])
                            dst = out_sb[it // 8][:, it % 8, :]
                            # alternate norm engines so neither serializes
                            # the tail (ACT is idle after the last exp)
                            if it % 2 == 0:
                                nc.scalar.activation(
                                    dst, acc[it][:, 0:H], AF.Copy, bias=0.0, scale=rc
                                )
                            else:
                                nc.vector.tensor_scalar_mul(
                                    dst, acc[it][:, 0:H], rc
                                )
                            if it % 8 == 7:
                                nc.sync.dma_start(
                                    out=out_ap[it // 8], in_=out_sb[it // 8]
                                )

                for jp in range(NQ):
                    vproj_quarter(jp)
                    av_phase(jp)


def build_nc():
    nc = bacc.Bacc(
        "TRN2", target_bir_lowering=False, debug=False, num_devices=N_CORES
    )
    ins = [
        nc.dram_tensor("qp", [NQ, P, DC, QS], BF16, kind="ExternalInput").ap(),
        nc.dram_tensor("kp", [NQ, P, DC, QS], BF16, kind="ExternalInput").ap(),
        nc.dram_tensor("vp", [NQ, P, DC, QS], BF16, kind="ExternalInput").ap(),
        nc.dram_tensor("wq", [P, DC, H], BF16, kind="ExternalInput").ap(),
        nc.dram_tensor("bq", [P, 1], F32, kind="ExternalInput").ap(),
        nc.dram_tensor("wk", [P, DC, H], BF16, kind="ExternalInput").ap(),
        nc.dram_tensor("bk", [P, 1], F32, kind="ExternalInput").ap(),
        nc.dram_tensor("wv", [P, DC, H], BF16, kind="ExternalInput").ap(),
        nc.dram_tensor("bv", [1, H], BF16, kind="ExternalInput").ap(),
    ]
    # packed [half, p, it_in_half, h]; host unpacks to [S, H]
    out_ap = nc.dram_tensor("out", [2, P, 8, H], F32, kind="ExternalOutput").ap()
    with tile.TileContext(nc) as tc:
        _build_kernel(tc, ins, out_ap)
    nc.compile()
    return nc


_NC_CACHE = None


def _get_nc():
    global _NC_CACHE
    if _NC_CACHE is None:
        _NC_CACHE = build_nc()
    return _NC_CACHE


def _pack_xt(x_f32, bf):
    """[S, D] f32 -> X^T packed [NQ, P, DC, QS] bf16 (8KB DMA lines)."""
    xt = np.ascontiguousarray(x_f32.astype(bf).T)          # [D, S]
    return np.ascontiguousarray(
        xt.reshape(DC, P, NQ, QS).transpose(2, 1, 0, 3)
    )


def _pack_w(w_f32, bf):
    """[D, H] f32 -> [P, DC, H] bf16 (2KB DMA lines)."""
    return np.ascontiguousarray(
        w_f32.astype(bf).reshape(DC, P, H).transpose(1, 0, 2)
    )


def _run(inputs, trace=False, **kw):
    import ml_dtypes

    nc = _get_nc()
    bf = np.dtype(ml_dtypes.bfloat16)
    q = np.asarray(inputs["query"], dtype=np.float32)
    k = np.asarray(inputs["key"], dtype=np.float32)
    v = np.asarray(inputs["value"], dtype=np.float32)
    shared = {
        "wq": _pack_w(np.asarray(inputs["Wq"], dtype=np.float32), bf),
        "wk": _pack_w(np.asarray(inputs["Wk"], dtype=np.float32), bf),
        "wv": _pack_w(np.asarray(inputs["Wv"], dtype=np.float32), bf),
        "bq": np.ascontiguousarray(
            np.asarray(inputs["bq"], dtype=np.float32).reshape(P, 1)
        ),
        "bk": np.ascontiguousarray(
            np.asarray(inputs["bk"], dtype=np.float32).reshape(P, 1)
        ),
        "bv": np.ascontiguousarray(
            np.asarray(inputs["bv"], dtype=np.float32).astype(bf).reshape(1, H)
        ),
    }
    in_maps = [
        {
            "qp": _pack_xt(q[c], bf),
            "kp": _pack_xt(k[c], bf),
            "vp": _pack_xt(v[c], bf),
            **shared,
        }
        for c in range(N_CORES)
    ]
    res = run_bass_kernel_spmd(nc, in_maps, list(range(N_CORES)), trace=trace, **kw)
    # unpack [2, P, 8, H] -> [S, H]: out[s,h] with s = 1024*half + 128*it + p
    out = np.stack(
        [
            res.results[c]["out"]
            .transpose(0, 2, 1, 3)
            .reshape(S, H)
            for c in range(N_CORES)
        ],
        axis=0,
    )
    return out.astype(np.float32), res


def kernel(**inputs) -> np.ndarray:
    out, _ = _run(inputs, trace=False)
    return out


if __name__ == "__main__":
    # smoke-build only
    build_nc()
    print("build ok")


# revision 10
# speedup vs baseline: 1.0491x; 1.0491x over previous
"""Bass/Tile kernel for a single attention head, data-parallel over B=8 on
8 TRN2 NeuronCores (one batch element per core, no collectives).

Per-core problem (S=2048, D=1024, H=128):
    q = Xq @ Wq + bq ; k = Xk @ Wk + bk ; v = Xv @ Wv + bv
    out = softmax(q k^T / sqrt(H)) v

v3 layout/schedule (PE contracts over the partition dim):
  - X^T built on the HOST (numpy transpose + bf16 cast + repack) so the
    PE spends zero cycles transposing inputs.  DMA lines are 2-8KB.
  - The ACT engine's exp stream (32 ops x ~1.3us) is the serial
    bottleneck; everything is scheduled around starting it early and
    never starving it:
      * weights/biases doorbells ride the GpSimd queue (a dma_start
        costs ~680ns of issue time; serializing 18 of them on Sync was
        delaying the first k bytes by ~5us),
      * k arrives in SIXTEENTHS so scores j0 starts after only ~2.8MB
        of byte-traffic, q half 0 right before it, v last.
  - scoresT [j, i] per j-tile; exp((1/sqrt H)x) PSUM->SBUF bf16 per
    (j-tile, i-half).
  - v projected DIRECTLY to natural [s, h] (stationary X_v^T s-slice,
    moving Wv chunk; bias via a 1-partition ones-row matmul); ones
    column makes AV emit numerator + row-sums in one accumulation.
  - AV is split by i-HALF: the lower 8 i-tiles only need i-half0 exp
    (done mid-kernel) + v, so they finish and store while the exp i1
    stream still runs; only the upper half's last j-phase (~2us) trails
    the final exp.  Accumulation in one SBUF f32 tile; drains on DVE;
    normalization = batched DVE reciprocal + muls on DVE/GpSimd (lower,
    ACT still busy) and ACT/DVE/GpSimd (upper, ACT idle).
  - Output leaves as [p, itile, h] packed halves (4KB DMA lines), host
    unpacks.
"""

import sys

if "/opt/trn_rl_repo" not in sys.path:
    sys.path.insert(0, "/opt/trn_rl_repo")

import numpy as np

import concourse.bass as bass
import concourse.tile as tile
from concourse import bacc, mybir
from concourse.bass_utils import run_bass_kernel_spmd

P = 128          # partitions
S = 2048         # sequence length (per core)
D = 1024         # input dim
H = 128          # head dim (Dq = Dk)
ST = S // P      # 16 s-tiles
DC = D // P      # 8 d-chunks
NQ = 4           # s-quarters
QS = S // NQ     # 512
N_CORES = 8

F32 = mybir.dt.float32
BF16 = mybir.dt.bfloat16
AF = mybir.ActivationFunctionType

SOFTMAX_SCALE = 1.0 / float(np.sqrt(H))


def _build_kernel(tc, ins, out_ap):
    nc = tc.nc
    (qp, kp, vp, wq_ap, bq_ap, wk_ap, bk_ap, wv_ap, bv_ap) = ins

    with (
        tc.tile_pool(name="consts", bufs=1) as consts,
        tc.tile_pool(name="xq", bufs=4) as xqp,
        tc.tile_pool(name="xk", bufs=16) as xkp,
        tc.tile_pool(name="xv", bufs=4) as xvp,
        tc.tile_pool(name="proj", bufs=1) as projp,
        tc.tile_pool(name="expp", bufs=1) as expp,
        tc.tile_pool(name="vext", bufs=1) as vexp,
        tc.tile_pool(name="accp", bufs=1) as accp,
        tc.tile_pool(name="outp", bufs=1) as outp,
    ):
        # ---- tiny consts (no DMA) ----
        warm_a = consts.tile([P, P], BF16, tag="warm_a")
        nc.gpsimd.memset(warm_a, 0.5)
        ones_row = consts.tile([P, P], BF16, tag="ones_row")
        nc.gpsimd.memset(ones_row, 1.0)
        warm_sink = nc.dram_tensor("warm_sink", [P, P], F32)

        # ---- load doorbells: weights/biases on GpSimd, X on Sync, both
        # in parallel; Sync carries the byte-priority order q_h0, k, q_h1,
        # v.  (Each dma_start costs ~680ns of queue issue time.) ----
        wq = consts.tile([P, DC, H], BF16, tag="wq")
        nc.gpsimd.dma_start(out=wq, in_=wq_ap)
        bq = consts.tile([P, 1], F32, tag="bq")
        nc.gpsimd.dma_start(out=bq, in_=bq_ap)
        wk = consts.tile([P, DC, H], BF16, tag="wk")
        nc.gpsimd.dma_start(out=wk, in_=wk_ap)
        bk = consts.tile([P, 1], F32, tag="bk")
        nc.gpsimd.dma_start(out=bk, in_=bk_ap)
        wv = consts.tile([P, DC, H], BF16, tag="wv")
        nc.gpsimd.dma_start(out=wv, in_=wv_ap)
        bvr = consts.tile([1, H], BF16, tag="bvr")
        nc.gpsimd.dma_start(out=bvr, in_=bv_ap)

        xq_tiles = []
        for nq in range(2):
            xt = xqp.tile([P, DC, QS], BF16, tag="xq", name=f"xq{nq}")
            nc.sync.dma_start(out=xt, in_=qp[nq])
            xq_tiles.append(xt)
        xk_tiles = []
        for t in range(ST):
            xt = xkp.tile([P, DC, P], BF16, tag="xk", name=f"xk{t}")
            nc.sync.dma_start(out=xt, in_=kp[t])
            xk_tiles.append(xt)
        for nq in range(2, NQ):
            xt = xqp.tile([P, DC, QS], BF16, tag="xq", name=f"xq{nq}")
            nc.sync.dma_start(out=xt, in_=qp[nq])
            xq_tiles.append(xt)
        xv_tiles = []
        for nq in range(NQ):
            xt = xvp.tile([P, DC, QS], BF16, tag="xv", name=f"xv{nq}")
            nc.sync.dma_start(out=xt, in_=vp[nq])
            xv_tiles.append(xt)

        # preload the ACT exp table set (~2.7us) during DMA dead time
        dummy = consts.tile([P, 1], F32, tag="dummy")
        nc.gpsimd.memset(dummy, 0.0)
        exp_sink = consts.tile([P, 1], BF16, tag="exp_sink")
        nc.scalar.activation(exp_sink, dummy, AF.Exp, bias=0.0, scale=1.0)

        # ---- persistent SBUF tiles ----
        qTq = [
            projp.tile([P, QS], BF16, tag=f"qT{i}", name=f"qT{i}")
            for i in range(NQ)
        ]
        kT16 = [
            projp.tile([P, P], BF16, tag=f"kT{t}", name=f"kT{t}")
            for t in range(ST)
        ]
        # exp tiles split by (i-half, j-quarter) so AV phases only wait
        # on the exp they actually read
        ex = [
            [
                expp.tile([P, 4, 1024], BF16, tag=f"ex{h}{jq}", name=f"ex{h}{jq}")
                for jq in range(NQ)
            ]
            for h in range(2)
        ]
        # v natural [s,h]+ones column, split by j-quarter
        vx = [
            vexp.tile([P, 4, H + 1], BF16, tag=f"vx{jq}", name=f"vx{jq}")
            for jq in range(NQ)
        ]
        for jq in range(NQ):
            nc.gpsimd.memset(vx[jq][:, :, H : H + 1], 1.0)
        # single acc tile: all drains ride DVE in-order anyway, and one
        # tile lets the reciprocals batch 8 denominators per op
        acc = accp.tile([P, ST, H + 4], F32, tag="acc")
        rc_all = accp.tile([P, ST], F32, tag="rc_all")
        out_sb = [
            outp.tile([P, 8, H], F32, tag=f"osb{hf}", name=f"osb{hf}")
            for hf in range(2)
        ]

        with (
            tc.tile_pool(name="psS", bufs=2, space="PSUM") as psS,   # 2x2 banks
            tc.tile_pool(name="psP", bufs=2, space="PSUM") as psP,   # 2x1 banks
            tc.tile_pool(name="psB", bufs=2, space="PSUM") as psB,   # 2x1 banks
        ):
            # ---- PE warm-up: HAM clock gate needs ~3.4us of sustained
            # PE activity to release full clock; burn it pre-data ----
            ps_w = psP.tile([P, QS], F32, tag="pp", name="ps_w")
            for _ in range(16):
                nc.tensor.matmul(
                    ps_w[:, 0:P], warm_a, warm_a, start=True, stop=True
                )
            warm_sb = consts.tile([P, P], F32, tag="warm_sb")
            nc.vector.tensor_copy(warm_sb, ps_w[:, 0:P])
            nc.sync.dma_start(out=warm_sink[:, :], in_=warm_sb)

            def proj_q(nq):
                ps = psP.tile([P, QS], F32, tag="pp")
                for dc in range(DC):
                    nc.tensor.matmul(
                        ps,
                        wq[:, dc, :],
                        xq_tiles[nq][:, dc, :],
                        start=(dc == 0),
                        stop=(dc == DC - 1),
                    )
                nc.vector.tensor_scalar_add(qTq[nq], ps, bq)

            def proj_k16(t):
                ps = psP.tile([P, QS], F32, tag="pp")
                for dc in range(DC):
                    nc.tensor.matmul(
                        ps[:, 0:P],
                        wk[:, dc, :],
                        xk_tiles[t][:, dc, :],
                        start=(dc == 0),
                        stop=(dc == DC - 1),
                    )
                nc.vector.tensor_scalar_add(kT16[t], ps[:, 0:P], bk)

            def scores_exp(jt, hf):
                pss = psS.tile([P, 1024], F32, tag="ps")
                for nb in range(2):
                    nc.tensor.matmul(
                        pss[:, nb * QS : (nb + 1) * QS],
                        kT16[jt],
                        qTq[2 * hf + nb],
                        start=True,
                        stop=True,
                    )
                nc.scalar.activation(
                    ex[hf][jt // 4][:, jt % 4, :],
                    pss,
                    AF.Exp,
                    bias=0.0,
                    scale=SOFTMAX_SCALE,
                )

            def vproj_quarter(jq):
                """v natural [s,h] for s-tiles 4jq..4jq+3; bias via a
                1-partition ones-row matmul into the same PSUM group."""
                xt = xv_tiles[jq]
                for st in range(4):
                    ps = psP.tile([P, QS], F32, tag="pp")
                    for dc in range(DC):
                        nc.tensor.matmul(
                            ps[:, 0:H],
                            xt[:, dc, st * P : (st + 1) * P],
                            wv[:, dc, :],
                            start=(dc == 0),
                            stop=False,
                        )
                    nc.tensor.matmul(
                        ps[:, 0:H],
                        ones_row[0:1, :],
                        bvr[0:1, :],
                        start=False,
                        stop=True,
                    )
                    nc.vector.tensor_copy(vx[jq][:, st, 0:H], ps[:, 0:H])

            def av_phase(ihalf, jp):
                """AV partials for i-tiles [8*ihalf, 8*ihalf+8) over
                j-tiles 4jp..4jp+3, accumulated into acc (SBUF f32)."""
                for k in range(8):
                    it = 8 * ihalf + k
                    po = psB.tile([P, H + 1], F32, tag="po")
                    for j in range(4):
                        nc.tensor.matmul(
                            po,
                            ex[ihalf][jp][:, j, k * P : (k + 1) * P],
                            vx[jp][:, j, :],
                            start=(j == 0),
                            stop=(j == 3),
                        )
                    if jp == 0:
                        nc.vector.tensor_copy(acc[:, it, 0 : H + 1], po)
                    else:
                        nc.vector.tensor_add(
                            acc[:, it, 0 : H + 1], acc[:, it, 0 : H + 1], po
                        )

            def norm_store(ihalf, engines):
                """Batch reciprocal, then scale+pack 8 i-tiles and DMA the
                half out.  `engines` alternates the muls."""
                i0 = 8 * ihalf
                nc.vector.reciprocal(
                    rc_all[:, i0 : i0 + 8], acc[:, i0 : i0 + 8, H : H + 1]
                )
                for k in range(8):
                    it = i0 + k
                    eng = engines[k % len(engines)]
                    dst = out_sb[ihalf][:, k, :]
                    if eng == "act":
                        nc.scalar.activation(
                            dst,
                            acc[:, it, 0:H],
                            AF.Copy,
                            bias=0.0,
                            scale=rc_all[:, it : it + 1],
                        )
                    elif eng == "dve":
                        nc.vector.tensor_scalar_mul(
                            dst, acc[:, it, 0:H], rc_all[:, it : it + 1]
                        )
                    else:
                        nc.gpsimd.tensor_scalar_mul(
                            dst, acc[:, it, 0:H], rc_all[:, it : it + 1]
                        )
                nc.sync.dma_start(out=out_ap[ihalf], in_=out_sb[ihalf])

            # ---- emission order == intended engine execution order ----
            proj_q(0)
            proj_q(1)
            for jt in range(ST):
                proj_k16(jt)
                scores_exp(jt, 0)
            proj_q(2)
            proj_q(3)
            for jt in range(ST):
                scores_exp(jt, 1)

            # AV: lower i-half chases v arrival (its exp is long done);
            # upper i-half chases the exp i1 stream.
            vproj_quarter(0)
            av_phase(0, 0)
            vproj_quarter(1)
            av_phase(0, 1)
            vproj_quarter(2)
            av_phase(0, 2)
            av_phase(1, 0)
            vproj_quarter(3)
            av_phase(0, 3)
            norm_store(0, ["dve", "gps"])   # ACT still busy with exp i1
            av_phase(1, 1)
            av_phase(1, 2)
            av_phase(1, 3)
            norm_store(1, ["act", "dve", "gps"])


def build_nc():
    nc = bacc.Bacc(
        "TRN2", target_bir_lowering=False, debug=False, num_devices=N_CORES
    )
    ins = [
        nc.dram_tensor("qp", [NQ, P, DC, QS], BF16, kind="ExternalInput").ap(),
        nc.dram_tensor("kp", [ST, P, DC, P], BF16, kind="ExternalInput").ap(),
        nc.dram_tensor("vp", [NQ, P, DC, QS], BF16, kind="ExternalInput").ap(),
        nc.dram_tensor("wq", [P, DC, H], BF16, kind="ExternalInput").ap(),
        nc.dram_tensor("bq", [P, 1], F32, kind="ExternalInput").ap(),
        nc.dram_tensor("wk", [P, DC, H], BF16, kind="ExternalInput").ap(),
        nc.dram_tensor("bk", [P, 1], F32, kind="ExternalInput").ap(),
        nc.dram_tensor("wv", [P, DC, H], BF16, kind="ExternalInput").ap(),
        nc.dram_tensor("bv", [1, H], BF16, kind="ExternalInput").ap(),
    ]
    # packed [half, p, it_in_half, h]; host unpacks to [S, H]
    out_ap = nc.dram_tensor("out", [2, P, 8, H], F32, kind="ExternalOutput").ap()
    with tile.TileContext(nc) as tc:
        _build_kernel(tc, ins, out_ap)
    nc.compile()
    return nc


_NC_CACHE = None


def _get_nc():
    global _NC_CACHE
    if _NC_CACHE is None:
        _NC_CACHE = build_nc()
    return _NC_CACHE


def _pack_xt(x_f32, bf, nblk):
    """[S, D] f32 -> X^T packed [nblk, P, DC, S//nblk] bf16."""
    xt = np.ascontiguousarray(x_f32.astype(bf).T)          # [D, S]
    return np.ascontiguousarray(
        xt.reshape(DC, P, nblk, S // nblk).transpose(2, 1, 0, 3)
    )


def _pack_w(w_f32, bf):
    """[D, H] f32 -> [P, DC, H] bf16 (2KB DMA lines)."""
    return np.ascontiguousarray(
        w_f32.astype(bf).reshape(DC, P, H).transpose(1, 0, 2)
    )


def _run(inputs, trace=False, **kw):
    import ml_dtypes

    nc = _get_nc()
    bf = np.dtype(ml_dtypes.bfloat16)
    q = np.asarray(inputs["query"], dtype=np.float32)
    k = np.asarray(inputs["key"], dtype=np.float32)
    v = np.asarray(inputs["value"], dtype=np.float32)
    shared = {
        "wq": _pack_w(np.asarray(inputs["Wq"], dtype=np.float32), bf),
        "wk": _pack_w(np.asarray(inputs["Wk"], dtype=np.float32), bf),
        "wv": _pack_w(np.asarray(inputs["Wv"], dtype=np.float32), bf),
        "bq": np.ascontiguousarray(
            np.asarray(inputs["bq"], dtype=np.float32).reshape(P, 1)
        ),
        "bk": np.ascontiguousarray(
            np.asarray(inputs["bk"], dtype=np.float32).reshape(P, 1)
        ),
        "bv": np.ascontiguousarray(
            np.asarray(inputs["bv"], dtype=np.float32).astype(bf).reshape(1, H)
        ),
    }
    in_maps = [
        {
            "qp": _pack_xt(q[c], bf, NQ),
            "kp": _pack_xt(k[c], bf, ST),
            "vp": _pack_xt(v[c], bf, NQ),
            **shared,
        }
        for c in range(N_CORES)
    ]
    res = run_bass_kernel_spmd(nc, in_maps, list(range(N_CORES)), trace=trace, **kw)
    # unpack [2, P, 8, H] -> [S, H]: s = 1024*half + 128*it + p
    out = np.stack(
        [
            res.results[c]["out"].transpose(0, 2, 1, 3).reshape(S, H)
            for c in range(N_CORES)
        ],
        axis=0,
    )
    return out.astype(np.float32), res


def kernel(**inputs) -> np.ndarray:
    out, _ = _run(inputs, trace=False)
    return out


if __name__ == "__main__":
    # smoke-build only
    build_nc()
    print("build ok")
